# revision 11
# baseline (speedup 1.0000x reference)
"""Trainium2 Bass kernel for nn_AtnScore (masked normalized-correlation softmax).

Math (per batch b):
  w = x2[b] viewed [C, N] (N = H*W, row-major), gram = w^T @ w  [N, N]
  a_l = 10 * (mask_l == 0) / max(||w[:,l]||, 1e-4)
  z[l, n] = a_l * gram[l, n]        (softmax over l, per column n)
  out[l, n] = max(softmax_l(z)[l, n] * (mask_l == 0), 1e-8)

Sharding: 8 cores = 4 batches x 2 column-halves (n in [0,2048) / [2048,4096)).
Each core computes z TRANSPOSED (partition = n-tile of its half, free = l) so
the softmax reduction runs along the free axis; the host gather transposes
back while upcasting.

The device handles exactly NU=2048 packed unmasked-l columns (one 4-bank
PSUM tile and ONE 2048-wide ACTIVATE per n-tile keeps the Act engine at its
structural floor).  The ≤44 unmasked columns beyond 2048 are computed on the
host (a [nex,C]@[C,N] BLAS sliver) and folded in exactly: the device also
returns its raw per-column sums s_main, and the host rescales the device
softmax by s_main/(s_main+s_extra) and emits the extra rows itself — no
approximation beyond the fp16/fp32 arithmetic already present.

No max-reduce: the exp bias is a host-computed rigorous Cauchy-Schwarz
bound U0(n) = ||x16_n|| * max_l ||a_l x16_l|| boosted by +79; with E in
fp32 the whole column fits fp32's ~175-nat range. exp overflow is
impossible by construction.

fp16 matmul operands run the PE at full rate; the output is fp16.
"""

import numpy as np

B, C, HH, WW = 4, 256, 64, 64
N = HH * WW          # 4096 (l dimension, also total n)
NU = 2048            # packed main unmasked-l columns == columns per core
P = 128              # partitions
KO = C // P          # 2 contraction tiles
NT = NU // P         # 16 n-tiles per core
BOOST = 79.0

_CACHE = {}


def _build():
    import concourse.bacc as bacc
    import concourse.tile as tile
    import concourse.mybir as mybir
    from concourse.bass import ds

    f32 = mybir.dt.float32
    f16 = mybir.dt.float16
    bf16 = mybir.dt.bfloat16
    Act = mybir.ActivationFunctionType

    nc = bacc.Bacc(None, target_bir_lowering=False)

    x2s_d = nc.dram_tensor("x2s16", [P, KO * NU], f16, kind="ExternalInput")
    x2n_d = nc.dram_tensor("x2n16", [P, KO * NU], f16, kind="ExternalInput")
    nb_d = nc.dram_tensor("nbias", [P, NT], f32, kind="ExternalInput")
    out_d = nc.dram_tensor("out", [NU, NU], bf16, kind="ExternalOutput")
    ssum_d = nc.dram_tensor("ssum", [P, NT], f32, kind="ExternalOutput")

    with tile.TileContext(nc) as tc:
        with tc.tile_pool(name="persist", bufs=1) as persist:
            x16s = persist.tile([P, KO, NU], f16)      # moving operand (packed l)
            x16n = persist.tile([P, KO, NU], f16)      # stationary operand
            nbias = persist.tile([P, NT], f32)
            ssall = persist.tile([P, NT], f32)         # raw softmax sums
            warm = persist.tile([P, 512], f16)
            # all input loads on ONE ring, priority-ordered: the first
            # activate is gated on the full x16s, so it transfers first;
            # then the x16n quarter for tiles 0-3; the rest streams behind
            nc.sync.dma_start(nbias[:], nb_d[:])
            for ch in range(2):
                nc.sync.dma_start(
                    x16s[:, :, ds(ch * (NU // 2), NU // 2)],
                    x2s_d[:].rearrange("p (ko n) -> p ko n", ko=KO)[
                        :, :, ds(ch * (NU // 2), NU // 2)])
            nc.sync.dma_start(
                x16n[:, :, ds(0, NU // 4)],
                x2n_d[:].rearrange("p (ko n) -> p ko n", ko=KO)[
                    :, :, ds(0, NU // 4)])
            nc.sync.dma_start(
                x16n[:, :, ds(NU // 4, 3 * NU // 4)],
                x2n_d[:].rearrange("p (ko n) -> p ko n", ko=KO)[
                    :, :, ds(NU // 4, 3 * NU // 4)])
            nc.vector.memset(warm[:], 0.0)

            with tc.tile_pool(name="zps", bufs=2, space="PSUM") as zps, \
                 tc.tile_pool(name="ebuf", bufs=3) as ebuf, \
                 tc.tile_pool(name="obuf", bufs=3) as obuf, \
                 tc.tile_pool(name="small", bufs=4) as small:
                for nt in range(NT):
                    z = zps.tile([P, NU], f32, name=f"z{nt}", tag="z")
                    if nt == 0:
                        # HAM warmup: ~3.4us of dummy PE work during the
                        # input-DMA wait so the real matmuls run at 2.4GHz.
                        # Writes are overwritten by the start=True matmul.
                        for _ in range(8):
                            nc.tensor.matmul(
                                z[:, ds(0, 512)], warm[:, ds(0, P)],
                                warm[:], start=True, stop=True)
                    for ko in range(KO):
                        for c4 in range(4):
                            nc.tensor.matmul(
                                z[:, ds(c4 * 512, 512)],
                                x16n[:, ko, ds(nt * P, P)],
                                x16s[:, ko, ds(c4 * 512, 512)],
                                start=(ko == 0), stop=(ko == KO - 1))
                    E = ebuf.tile([P, NU], bf16, name=f"E{nt}", tag="E")
                    nc.scalar.activation(
                        E[:], z[:], Act.Exp,
                        bias=nbias[:, ds(nt, 1)], scale=1.0,
                        accum_out=ssall[:, ds(nt, 1)])

                    stot = small.tile([P, 1], f32, name=f"st{nt}", tag="st")
                    nc.vector.tensor_scalar_max(
                        stot[:], ssall[:, ds(nt, 1)], 1e-30)
                    rtot = small.tile([P, 1], f32, name=f"rt{nt}", tag="rt")
                    nc.vector.reciprocal_approx_fast(rtot[:], stot[:])

                    o16 = obuf.tile([P, NU], bf16, name=f"o{nt}", tag="o")
                    if nt < NT - 1:
                        nc.vector.tensor_scalar_mul(o16[:], E[:], rtot[:])
                        nc.sync.dma_start(out_d[ds(nt * P, P), :], o16[:])
                    else:
                        # split the last tile so its DMA overlaps the mul
                        for ch in range(2):
                            sl = ds(ch * (NU // 2), NU // 2)
                            nc.vector.tensor_scalar_mul(
                                o16[:, sl], E[:, sl], rtot[:])
                            nc.sync.dma_start(
                                out_d[ds(nt * P, P), sl], o16[:, sl])
                nc.sync.dma_start(ssum_d[:], ssall[:])
    nc.finalize()
    return nc


def _get_nc():
    if "nc" not in _CACHE:
        _CACHE["nc"] = _build()
    return _CACHE["nc"]


def _ensure_ntff_hook():
    """bass_utils under axon imports antenv.axon_hooks for trace=True; this
    image's antenv lacks it. Install a stub wired to the boot ctypes hook."""
    import sys
    import types
    try:
        import antenv.axon_hooks  # noqa: F401
        return
    except ImportError:
        pass
    mod = types.ModuleType("antenv.axon_hooks")
    _h = [None]
    mod.set_axon_ntff_profile_hook = lambda hook: _h.__setitem__(0, hook)
    mod.get_axon_ntff_profile_hook = lambda: _h[0]
    sys.modules["antenv.axon_hooks"] = mod
    try:
        import antenv
        antenv.axon_hooks = mod
    except ImportError:
        pass
    try:
        from trn_agent_boot.trn_boot import _ntff_profile_via_ctypes
        hook = _ntff_profile_via_ctypes("/opt/axon/libaxon_pjrt.so")
        if hook is not None:
            mod.set_axon_ntff_profile_hook(hook)
    except Exception:
        pass


def _interleave(arr16):
    """[C, NU] -> [P, KO*NU] so the SBUF tile [P, KO, NU] maps c = ko*P + p."""
    return np.ascontiguousarray(
        arr16.reshape(KO, P, NU).transpose(1, 0, 2).reshape(P, KO * NU))


def kernel(x2: np.ndarray, mask: np.ndarray) -> np.ndarray:
    from concourse.bass_utils import run_bass_kernel_spmd
    import os

    nc = _get_nc()
    x2 = np.ascontiguousarray(x2, dtype=np.float32)
    mask = np.ascontiguousarray(mask, dtype=np.float32)

    in_maps = []
    host = []  # per-core host-side state for the gather
    for core in range(8):
        b, h = core // 2, core % 2
        xb = x2[b].reshape(C, N)
        mb = mask[b].reshape(N)
        idx = np.flatnonzero(mb == 0.0)
        main_idx, extra_idx = idx[:NU], idx[NU:]
        sumsq = np.einsum("cn,cn->n", xb, xb, dtype=np.float64)
        norm = np.sqrt(sumsq).astype(np.float32)
        a = (10.0 / np.maximum(norm, 1e-4)).astype(np.float32)
        x2s16 = np.zeros((C, NU), dtype=np.float16)
        x2s16[:, :len(main_idx)] = (
            xb[:, main_idx] * a[None, main_idx]).astype(np.float16)
        x2n16 = np.ascontiguousarray(
            xb[:, h * NU:(h + 1) * NU]).astype(np.float16)
        # rigorous C-S bound on the f16 dot products, as the exp bias
        n16 = np.linalg.norm(x2n16.astype(np.float32), axis=0)
        y16max = float(np.linalg.norm(x2s16.astype(np.float32), axis=0).max())
        u0 = n16 * y16max * 1.001 + 0.5
        bias = (BOOST - u0).astype(np.float32)  # [NU] for local n
        nbias = bias.reshape(NT, P).T  # [P, NT]
        # host-side extra columns: raw scores for this core's n-half
        if len(extra_idx):
            wl = (xb[:, extra_idx] * a[None, extra_idx]).T  # [nex, C]
            z_extra = wl @ xb[:, h * NU:(h + 1) * NU]       # [nex, NU] f32
            e_extra = np.exp(z_extra.astype(np.float64) + bias[None, :])
            s_extra = e_extra.sum(axis=0)                    # [NU]
        else:
            e_extra, s_extra = None, 0.0
        host.append((main_idx, extra_idx, e_extra, s_extra))
        in_maps.append({
            "x2s16": _interleave(x2s16),
            "x2n16": _interleave(x2n16),
            "nbias": np.ascontiguousarray(nbias),
        })

    trace = bool(int(os.environ.get("ATN_TRACE", "0")))
    if trace:
        _ensure_ntff_hook()
    res = run_bass_kernel_spmd(nc, in_maps, list(range(8)), trace=trace)
    if trace and res.exec_time_ns is not None:
        print(f"HW exec time: {res.exec_time_ns} ns")
        _CACHE["last_exec_ns"] = res.exec_time_ns
        _CACHE["last_results"] = res

    out = np.full((B, N, N), 1e-8, dtype=np.float32)
    for core in range(8):
        b, h = core // 2, core % 2
        main_idx, extra_idx, e_extra, s_extra = host[core]
        s_main = res.results[core]["ssum"].T.ravel().astype(np.float64)  # [NU]
        s_tot = s_main + s_extra
        cols = slice(h * NU, (h + 1) * NU)
        # main rows: device softmax (normalized by s_main) rescaled exactly
        ratio = (s_main / np.maximum(s_tot, 1e-300)).astype(np.float32)
        dev = res.results[core]["out"][:, :len(main_idx)].astype(np.float32)
        dev *= ratio[:, None]
        np.maximum(dev, 1e-8, out=dev)
        out[b][main_idx, cols] = dev.T
        # extra rows: host exp over the shared bias, same denominator
        if len(extra_idx):
            ex = (e_extra / np.maximum(s_tot, 1e-300)[None, :]).astype(
                np.float32)
            np.maximum(ex, 1e-8, out=ex)
            out[b][extra_idx, cols] = ex
    return out.reshape(B, N, HH, WW)


# revision 13
# speedup vs baseline: 1.0649x; 1.0649x over previous
"""Trainium2 Bass kernel for nn_AtnScore (masked normalized-correlation softmax).

Math (per batch b):
  w = x2[b] viewed [C, N] (N = H*W, row-major), gram = w^T @ w  [N, N]
  a_l = 10 * (mask_l == 0) / max(||w[:,l]||, 1e-4)
  z[l, n] = a_l * gram[l, n]        (softmax over l, per column n)
  out[l, n] = max(softmax_l(z)[l, n] * (mask_l == 0), 1e-8)

Sharding: 8 cores = 4 batches x 2 column-halves (n in [0,2048) / [2048,4096)).
Each core computes z TRANSPOSED (partition = n-tile of its half, free = l) so
the softmax reduction runs along the free axis; the host gather transposes
back while upcasting.

The device handles exactly NU=2048 packed unmasked-l columns (one 4-bank
PSUM tile and ONE 2048-wide ACTIVATE per n-tile keeps the Act engine at its
structural floor).  The ≤44 unmasked columns beyond 2048 are computed on the
host (a [nex,C]@[C,N] BLAS sliver) and folded in exactly: the device also
returns its raw per-column sums s_main, and the host rescales the device
softmax by s_main/(s_main+s_extra) and emits the extra rows itself — no
approximation beyond the fp16/fp32 arithmetic already present.

No max-reduce: the exp bias is a host-computed rigorous Cauchy-Schwarz
bound U0(n) = ||x16_n|| * max_l ||a_l x16_l|| boosted by +79; with E in
fp32 the whole column fits fp32's ~175-nat range. exp overflow is
impossible by construction.

fp16 matmul operands run the PE at full rate; the output is fp16.
"""

import numpy as np

B, C, HH, WW = 4, 256, 64, 64
N = HH * WW          # 4096 (l dimension, also total n)
NU = 2048            # packed main unmasked-l columns == columns per core
P = 128              # partitions
KO = C // P          # 2 contraction tiles
NT = NU // P         # 16 n-tiles per core
BOOST = 79.0

_CACHE = {}


def _build():
    import concourse.bacc as bacc
    import concourse.tile as tile
    import concourse.mybir as mybir
    from concourse.bass import ds

    f32 = mybir.dt.float32
    f16 = mybir.dt.float16
    bf16 = mybir.dt.bfloat16
    Act = mybir.ActivationFunctionType

    nc = bacc.Bacc(None, target_bir_lowering=False)

    x2s_d = nc.dram_tensor("x2s16", [P, KO * NU], f16, kind="ExternalInput")
    x2n_d = nc.dram_tensor("x2n16", [P, KO * NU], f16, kind="ExternalInput")
    nb_d = nc.dram_tensor("nbias", [P, NT], f32, kind="ExternalInput")
    out_d = nc.dram_tensor("out", [NU, NU], bf16, kind="ExternalOutput")
    ssum_d = nc.dram_tensor("ssum", [P, NT], f32, kind="ExternalOutput")

    with tile.TileContext(nc) as tc:
        with tc.tile_pool(name="persist", bufs=1) as persist:
            x16s = persist.tile([P, KO, NU], f16)      # moving operand (packed l)
            x16n = persist.tile([P, KO, NU], f16)      # stationary operand
            nbias = persist.tile([P, NT], f32)
            ssall = persist.tile([P, NT], f32)         # raw softmax sums
            # all input loads on ONE ring, priority-ordered so the pieces
            # gating the first matmuls/activate land first: xs half 1 ->
            # xn quarter 1 (tiles 0-3 stationary) -> xs half 2 -> xn rest
            xs_r = x2s_d[:].rearrange("p (ko n) -> p ko n", ko=KO)
            xn_r = x2n_d[:].rearrange("p (ko n) -> p ko n", ko=KO)
            nc.sync.dma_start(nbias[:], nb_d[:])
            nc.sync.dma_start(x16s[:, :, ds(0, NU // 2)],
                              xs_r[:, :, ds(0, NU // 2)])
            nc.sync.dma_start(x16n[:, :, ds(0, NU // 4)],
                              xn_r[:, :, ds(0, NU // 4)])
            nc.sync.dma_start(x16s[:, :, ds(NU // 2, NU // 2)],
                              xs_r[:, :, ds(NU // 2, NU // 2)])
            nc.sync.dma_start(x16n[:, :, ds(NU // 4, 3 * NU // 4)],
                              xn_r[:, :, ds(NU // 4, 3 * NU // 4)])

            with tc.tile_pool(name="zps", bufs=2, space="PSUM") as zps, \
                 tc.tile_pool(name="ebuf", bufs=3) as ebuf, \
                 tc.tile_pool(name="obuf", bufs=3) as obuf, \
                 tc.tile_pool(name="small", bufs=4) as small:
                for nt in range(NT):
                    z = zps.tile([P, NU], f32, name=f"z{nt}", tag="z")
                    # c4-outer: chunks 0-1 only need xs half 1, so they
                    # start while xs half 2 is still in flight
                    for c4 in range(4):
                        for ko in range(KO):
                            nc.tensor.matmul(
                                z[:, ds(c4 * 512, 512)],
                                x16n[:, ko, ds(nt * P, P)],
                                x16s[:, ko, ds(c4 * 512, 512)],
                                start=(ko == 0), stop=(ko == KO - 1))
                    E = ebuf.tile([P, NU], bf16, name=f"E{nt}", tag="E")
                    nc.scalar.activation(
                        E[:], z[:], Act.Exp,
                        bias=nbias[:, ds(nt, 1)], scale=1.0,
                        accum_out=ssall[:, ds(nt, 1)])

                    stot = small.tile([P, 1], f32, name=f"st{nt}", tag="st")
                    nc.vector.tensor_scalar_max(
                        stot[:], ssall[:, ds(nt, 1)], 1e-30)
                    rtot = small.tile([P, 1], f32, name=f"rt{nt}", tag="rt")
                    nc.vector.reciprocal_approx_fast(rtot[:], stot[:])

                    o16 = obuf.tile([P, NU], bf16, name=f"o{nt}", tag="o")
                    if nt < NT - 1:
                        nc.vector.tensor_scalar_mul(o16[:], E[:], rtot[:])
                        nc.sync.dma_start(out_d[ds(nt * P, P), :], o16[:])
                    else:
                        # split the last tile so its DMA overlaps the mul
                        for ch in range(2):
                            sl = ds(ch * (NU // 2), NU // 2)
                            nc.vector.tensor_scalar_mul(
                                o16[:, sl], E[:, sl], rtot[:])
                            nc.sync.dma_start(
                                out_d[ds(nt * P, P), sl], o16[:, sl])
                nc.sync.dma_start(ssum_d[:], ssall[:])
    nc.finalize()
    return nc


def _get_nc():
    if "nc" not in _CACHE:
        _CACHE["nc"] = _build()
    return _CACHE["nc"]


def _ensure_ntff_hook():
    """bass_utils under axon imports antenv.axon_hooks for trace=True; this
    image's antenv lacks it. Install a stub wired to the boot ctypes hook."""
    import sys
    import types
    try:
        import antenv.axon_hooks  # noqa: F401
        return
    except ImportError:
        pass
    mod = types.ModuleType("antenv.axon_hooks")
    _h = [None]
    mod.set_axon_ntff_profile_hook = lambda hook: _h.__setitem__(0, hook)
    mod.get_axon_ntff_profile_hook = lambda: _h[0]
    sys.modules["antenv.axon_hooks"] = mod
    try:
        import antenv
        antenv.axon_hooks = mod
    except ImportError:
        pass
    try:
        from trn_agent_boot.trn_boot import _ntff_profile_via_ctypes
        hook = _ntff_profile_via_ctypes("/opt/axon/libaxon_pjrt.so")
        if hook is not None:
            mod.set_axon_ntff_profile_hook(hook)
    except Exception:
        pass


def _interleave(arr16):
    """[C, NU] -> [P, KO*NU] so the SBUF tile [P, KO, NU] maps c = ko*P + p."""
    return np.ascontiguousarray(
        arr16.reshape(KO, P, NU).transpose(1, 0, 2).reshape(P, KO * NU))


def kernel(x2: np.ndarray, mask: np.ndarray) -> np.ndarray:
    from concourse.bass_utils import run_bass_kernel_spmd
    import os

    nc = _get_nc()
    x2 = np.ascontiguousarray(x2, dtype=np.float32)
    mask = np.ascontiguousarray(mask, dtype=np.float32)

    in_maps = []
    host = []  # per-core host-side state for the gather
    for core in range(8):
        b, h = core // 2, core % 2
        xb = x2[b].reshape(C, N)
        mb = mask[b].reshape(N)
        idx = np.flatnonzero(mb == 0.0)
        main_idx, extra_idx = idx[:NU], idx[NU:]
        sumsq = np.einsum("cn,cn->n", xb, xb, dtype=np.float64)
        norm = np.sqrt(sumsq).astype(np.float32)
        a = (10.0 / np.maximum(norm, 1e-4)).astype(np.float32)
        x2s16 = np.zeros((C, NU), dtype=np.float16)
        x2s16[:, :len(main_idx)] = (
            xb[:, main_idx] * a[None, main_idx]).astype(np.float16)
        x2n16 = np.ascontiguousarray(
            xb[:, h * NU:(h + 1) * NU]).astype(np.float16)
        # rigorous C-S bound on the f16 dot products, as the exp bias
        n16 = np.linalg.norm(x2n16.astype(np.float32), axis=0)
        y16max = float(np.linalg.norm(x2s16.astype(np.float32), axis=0).max())
        u0 = n16 * y16max * 1.001 + 0.5
        bias = (BOOST - u0).astype(np.float32)  # [NU] for local n
        nbias = bias.reshape(NT, P).T  # [P, NT]
        # host-side extra columns: raw scores for this core's n-half
        if len(extra_idx):
            wl = (xb[:, extra_idx] * a[None, extra_idx]).T  # [nex, C]
            z_extra = wl @ xb[:, h * NU:(h + 1) * NU]       # [nex, NU] f32
            e_extra = np.exp(z_extra.astype(np.float64) + bias[None, :])
            s_extra = e_extra.sum(axis=0)                    # [NU]
        else:
            e_extra, s_extra = None, 0.0
        host.append((main_idx, extra_idx, e_extra, s_extra))
        in_maps.append({
            "x2s16": _interleave(x2s16),
            "x2n16": _interleave(x2n16),
            "nbias": np.ascontiguousarray(nbias),
        })

    trace = bool(int(os.environ.get("ATN_TRACE", "0")))
    if trace:
        _ensure_ntff_hook()
    res = run_bass_kernel_spmd(nc, in_maps, list(range(8)), trace=trace)
    if trace and res.exec_time_ns is not None:
        print(f"HW exec time: {res.exec_time_ns} ns")
        _CACHE["last_exec_ns"] = res.exec_time_ns
        _CACHE["last_results"] = res

    out = np.full((B, N, N), 1e-8, dtype=np.float32)
    for core in range(8):
        b, h = core // 2, core % 2
        main_idx, extra_idx, e_extra, s_extra = host[core]
        s_main = res.results[core]["ssum"].T.ravel().astype(np.float64)  # [NU]
        s_tot = s_main + s_extra
        cols = slice(h * NU, (h + 1) * NU)
        # main rows: device softmax (normalized by s_main) rescaled exactly
        ratio = (s_main / np.maximum(s_tot, 1e-300)).astype(np.float32)
        dev = res.results[core]["out"][:, :len(main_idx)].astype(np.float32)
        dev *= ratio[:, None]
        np.maximum(dev, 1e-8, out=dev)
        out[b][main_idx, cols] = dev.T
        # extra rows: host exp over the shared bias, same denominator
        if len(extra_idx):
            ex = (e_extra / np.maximum(s_tot, 1e-300)[None, :]).astype(
                np.float32)
            np.maximum(ex, 1e-8, out=ex)
            out[b][extra_idx, cols] = ex
    return out.reshape(B, N, HH, WW)


# revision 15
# speedup vs baseline: 1.0763x; 1.0107x over previous
"""Trainium2 Bass kernel for nn_AtnScore (masked normalized-correlation softmax).

Math (per batch b):
  w = x2[b] viewed [C, N] (N = H*W, row-major), gram = w^T @ w  [N, N]
  a_l = 10 * (mask_l == 0) / max(||w[:,l]||, 1e-4)
  z[l, n] = a_l * gram[l, n]        (softmax over l, per column n)
  out[l, n] = max(softmax_l(z)[l, n] * (mask_l == 0), 1e-8)

Sharding: 8 cores = 4 batches x 2 column-halves (n in [0,2048) / [2048,4096)).
Each core computes z TRANSPOSED (partition = n-tile of its half, free = l) so
the softmax reduction runs along the free axis; the host gather transposes
back while upcasting.

The device handles exactly NU=2048 packed unmasked-l columns (one 4-bank
PSUM tile and ONE 2048-wide ACTIVATE per n-tile keeps the Act engine at its
structural floor).  The ≤44 unmasked columns beyond 2048 are computed on the
host (a [nex,C]@[C,N] BLAS sliver) and folded in exactly: the device also
returns its raw per-column sums s_main, and the host rescales the device
softmax by s_main/(s_main+s_extra) and emits the extra rows itself — no
approximation beyond the fp16/fp32 arithmetic already present.

No max-reduce: the exp bias is a host-computed rigorous Cauchy-Schwarz
bound U0(n) = ||x16_n|| * max_l ||a_l x16_l|| boosted by +79; with E in
fp32 the whole column fits fp32's ~175-nat range. exp overflow is
impossible by construction.

fp16 matmul operands run the PE at full rate; the output is fp16.
"""

import numpy as np

B, C, HH, WW = 4, 256, 64, 64
N = HH * WW          # 4096 (l dimension, also total n)
NU = 2048            # packed main unmasked-l columns == columns per core
P = 128              # partitions
KO = C // P          # 2 contraction tiles
NT = NU // P         # 16 n-tiles per core
BOOST = 79.0

_CACHE = {}


def _build():
    import concourse.bacc as bacc
    import concourse.tile as tile
    import concourse.mybir as mybir
    from concourse.bass import ds

    f32 = mybir.dt.float32
    f16 = mybir.dt.float16
    bf16 = mybir.dt.bfloat16
    Act = mybir.ActivationFunctionType

    nc = bacc.Bacc(None, target_bir_lowering=False)

    x2s_d = nc.dram_tensor("x2s16", [P, KO * NU], f16, kind="ExternalInput")
    x2n_d = nc.dram_tensor("x2n16", [P, KO * NU], f16, kind="ExternalInput")
    nb_d = nc.dram_tensor("nbias", [P, NT], f32, kind="ExternalInput")
    out_d = nc.dram_tensor("out", [NU, NU], bf16, kind="ExternalOutput")
    ssum_d = nc.dram_tensor("ssum", [P, NT], f32, kind="ExternalOutput")

    with tile.TileContext(nc) as tc:
        with tc.tile_pool(name="persist", bufs=1) as persist:
            x16s = persist.tile([P, KO, NU], f16)      # moving operand (packed l)
            x16n = persist.tile([P, KO, NU], f16)      # stationary operand
            nbias = persist.tile([P, NT], f32)
            ssall = persist.tile([P, NT], f32)         # raw softmax sums
            # all input loads on ONE ring, priority-ordered so the pieces
            # gating the first matmuls/activate land first.  ko-major
            # chunks are fully contiguous per partition (4KB segments):
            # xs ko0 -> xn ko0 q1 -> xs ko1 -> xn ko1 q1 -> xn rest
            xs_r = x2s_d[:].rearrange("p (ko n) -> p ko n", ko=KO)
            xn_r = x2n_d[:].rearrange("p (ko n) -> p ko n", ko=KO)
            nc.sync.dma_start(nbias[:], nb_d[:])
            nc.sync.dma_start(x16s[:, 0, :], xs_r[:, 0, :])
            nc.sync.dma_start(x16n[:, 0, ds(0, NU // 4)],
                              xn_r[:, 0, ds(0, NU // 4)])
            nc.sync.dma_start(x16s[:, 1, :], xs_r[:, 1, :])
            nc.sync.dma_start(x16n[:, 1, ds(0, NU // 4)],
                              xn_r[:, 1, ds(0, NU // 4)])
            for ko in range(KO):
                nc.sync.dma_start(
                    x16n[:, ko, ds(NU // 4, 3 * NU // 4)],
                    xn_r[:, ko, ds(NU // 4, 3 * NU // 4)])

            with tc.tile_pool(name="zps", bufs=2, space="PSUM") as zps, \
                 tc.tile_pool(name="ebuf", bufs=3) as ebuf, \
                 tc.tile_pool(name="obuf", bufs=3) as obuf, \
                 tc.tile_pool(name="small", bufs=4) as small:
                for nt in range(NT):
                    z = zps.tile([P, NU], f32, name=f"z{nt}", tag="z")
                    # ko-outer: the ko=0 pass only needs the first input
                    # chunk, so it runs while the ko=1 data is in flight
                    for ko in range(KO):
                        for c4 in range(4):
                            nc.tensor.matmul(
                                z[:, ds(c4 * 512, 512)],
                                x16n[:, ko, ds(nt * P, P)],
                                x16s[:, ko, ds(c4 * 512, 512)],
                                start=(ko == 0), stop=(ko == KO - 1))
                    E = ebuf.tile([P, NU], bf16, name=f"E{nt}", tag="E")
                    nc.scalar.activation(
                        E[:], z[:], Act.Exp,
                        bias=nbias[:, ds(nt, 1)], scale=1.0,
                        accum_out=ssall[:, ds(nt, 1)])

                    stot = small.tile([P, 1], f32, name=f"st{nt}", tag="st")
                    nc.vector.tensor_scalar_max(
                        stot[:], ssall[:, ds(nt, 1)], 1e-30)
                    rtot = small.tile([P, 1], f32, name=f"rt{nt}", tag="rt")
                    nc.vector.reciprocal_approx_fast(rtot[:], stot[:])

                    o16 = obuf.tile([P, NU], bf16, name=f"o{nt}", tag="o")
                    if nt < NT - 1:
                        nc.vector.tensor_scalar_mul(o16[:], E[:], rtot[:])
                        nc.sync.dma_start(out_d[ds(nt * P, P), :], o16[:])
                    else:
                        # split the last tile so its DMA overlaps the mul
                        for ch in range(2):
                            sl = ds(ch * (NU // 2), NU // 2)
                            nc.vector.tensor_scalar_mul(
                                o16[:, sl], E[:, sl], rtot[:])
                            nc.sync.dma_start(
                                out_d[ds(nt * P, P), sl], o16[:, sl])
                nc.sync.dma_start(ssum_d[:], ssall[:])
    nc.finalize()
    return nc


def _get_nc():
    if "nc" not in _CACHE:
        _CACHE["nc"] = _build()
    return _CACHE["nc"]


def _ensure_ntff_hook():
    """bass_utils under axon imports antenv.axon_hooks for trace=True; this
    image's antenv lacks it. Install a stub wired to the boot ctypes hook."""
    import sys
    import types
    try:
        import antenv.axon_hooks  # noqa: F401
        return
    except ImportError:
        pass
    mod = types.ModuleType("antenv.axon_hooks")
    _h = [None]
    mod.set_axon_ntff_profile_hook = lambda hook: _h.__setitem__(0, hook)
    mod.get_axon_ntff_profile_hook = lambda: _h[0]
    sys.modules["antenv.axon_hooks"] = mod
    try:
        import antenv
        antenv.axon_hooks = mod
    except ImportError:
        pass
    try:
        from trn_agent_boot.trn_boot import _ntff_profile_via_ctypes
        hook = _ntff_profile_via_ctypes("/opt/axon/libaxon_pjrt.so")
        if hook is not None:
            mod.set_axon_ntff_profile_hook(hook)
    except Exception:
        pass


def _interleave(arr16):
    """[C, NU] -> [P, KO*NU] so the SBUF tile [P, KO, NU] maps c = ko*P + p."""
    return np.ascontiguousarray(
        arr16.reshape(KO, P, NU).transpose(1, 0, 2).reshape(P, KO * NU))


def kernel(x2: np.ndarray, mask: np.ndarray) -> np.ndarray:
    from concourse.bass_utils import run_bass_kernel_spmd
    import os

    nc = _get_nc()
    x2 = np.ascontiguousarray(x2, dtype=np.float32)
    mask = np.ascontiguousarray(mask, dtype=np.float32)

    in_maps = []
    host = []  # per-core host-side state for the gather
    for core in range(8):
        b, h = core // 2, core % 2
        xb = x2[b].reshape(C, N)
        mb = mask[b].reshape(N)
        idx = np.flatnonzero(mb == 0.0)
        main_idx, extra_idx = idx[:NU], idx[NU:]
        sumsq = np.einsum("cn,cn->n", xb, xb, dtype=np.float64)
        norm = np.sqrt(sumsq).astype(np.float32)
        a = (10.0 / np.maximum(norm, 1e-4)).astype(np.float32)
        x2s16 = np.zeros((C, NU), dtype=np.float16)
        x2s16[:, :len(main_idx)] = (
            xb[:, main_idx] * a[None, main_idx]).astype(np.float16)
        x2n16 = np.ascontiguousarray(
            xb[:, h * NU:(h + 1) * NU]).astype(np.float16)
        # rigorous C-S bound on the f16 dot products, as the exp bias
        n16 = np.linalg.norm(x2n16.astype(np.float32), axis=0)
        y16max = float(np.linalg.norm(x2s16.astype(np.float32), axis=0).max())
        u0 = n16 * y16max * 1.001 + 0.5
        bias = (BOOST - u0).astype(np.float32)  # [NU] for local n
        nbias = bias.reshape(NT, P).T  # [P, NT]
        # host-side extra columns: raw scores for this core's n-half
        if len(extra_idx):
            wl = (xb[:, extra_idx] * a[None, extra_idx]).T  # [nex, C]
            z_extra = wl @ xb[:, h * NU:(h + 1) * NU]       # [nex, NU] f32
            e_extra = np.exp(z_extra.astype(np.float64) + bias[None, :])
            s_extra = e_extra.sum(axis=0)                    # [NU]
        else:
            e_extra, s_extra = None, 0.0
        host.append((main_idx, extra_idx, e_extra, s_extra))
        in_maps.append({
            "x2s16": _interleave(x2s16),
            "x2n16": _interleave(x2n16),
            "nbias": np.ascontiguousarray(nbias),
        })

    trace = bool(int(os.environ.get("ATN_TRACE", "0")))
    if trace:
        _ensure_ntff_hook()
    res = run_bass_kernel_spmd(nc, in_maps, list(range(8)), trace=trace)
    if trace and res.exec_time_ns is not None:
        print(f"HW exec time: {res.exec_time_ns} ns")
        _CACHE["last_exec_ns"] = res.exec_time_ns
        _CACHE["last_results"] = res

    out = np.full((B, N, N), 1e-8, dtype=np.float32)
    for core in range(8):
        b, h = core // 2, core % 2
        main_idx, extra_idx, e_extra, s_extra = host[core]
        s_main = res.results[core]["ssum"].T.ravel().astype(np.float64)  # [NU]
        s_tot = s_main + s_extra
        cols = slice(h * NU, (h + 1) * NU)
        # main rows: device softmax (normalized by s_main) rescaled exactly
        ratio = (s_main / np.maximum(s_tot, 1e-300)).astype(np.float32)
        dev = res.results[core]["out"][:, :len(main_idx)].astype(np.float32)
        dev *= ratio[:, None]
        np.maximum(dev, 1e-8, out=dev)
        out[b][main_idx, cols] = dev.T
        # extra rows: host exp over the shared bias, same denominator
        if len(extra_idx):
            ex = (e_extra / np.maximum(s_tot, 1e-300)[None, :]).astype(
                np.float32)
            np.maximum(ex, 1e-8, out=ex)
            out[b][extra_idx, cols] = ex
    return out.reshape(B, N, HH, WW)
